# revision 30
# baseline (speedup 1.0000x reference)
"""Trainium2 Bass kernel for nn_AttentionBlock (B=2, S=2048, D=1024, H=16).

Sharding: 8 cores = data-parallel over batch (2) x tensor-parallel over
head groups (4 heads per core).  Each core computes its 4 heads'
attention plus its slice of the qkv / out projections; the host sums the
4 per-batch partial outputs and adds b_out.

All matmul operands and the output DMA are bf16 (PSUM accumulation and
the softmax stay f32); absmax-relative error lands ~7e-3 against the
2e-2 gate.  HW-trace-driven design notes (325us baseline -> ~240us):

  - The kernel is PE-throughput-bound: ~850 matmuls x ~230ns (the PE
    streams 1 moving column/cycle at ~2.0-2.4GHz; LDWEIGHTS overlap).
    ACT's 128 softmax-exp tiles ([128,1024] @ ~1.03us) total ~140us.
  - PE is strictly in-order, so the per-step emission is
    exp(t); S(t+1); PV(t-1); fillers -- the PV lags its exp by one step
    so nothing in the PE stream ever waits on the currently-running exp
    (otherwise every filler chain lands on the ACT critical path).
  - Filler chains (qkv/final projections) ride in the per-step PE slack;
    v/k0 chains are phase-0-locked by dataflow, everything else is
    spread just-ahead-of-deadline.  Phases run pair-0 for all query
    chunks first, then pair-1, which spreads the k1/q1/final work.
  - normalize: DVE reciprocal costs 3.3us per [1,512] -- use
    reciprocal_approx_fast (0.6us) on an SBUF copy of the rowsum (the
    custom DVE op misreads PSUM partition 64 directly), broadcast it
    across the 64 hd-partitions with a rank-1 f32r PE matmul (gpsimd
    partition_broadcast costs ~2.5us), multiply on DVE.
  - weights/consts load once outside the bench rep loop (resident);
    per-iteration DMA is one descriptor per 512-column xT chunk (the
    ring is issue-limited at ~0.66us/descriptor); output rides the
    gpsimd HWDGE ring in bf16.

Per-core layout:
  - host passes x[b].T so the contraction dim (d) is the partition dim
  - q,k computed transposed [e, s]; v computed natural [s, hd]
  - S^T[j,i] = k_h q_h^T, two heads packed in the PE array (row groups
    (0,0)/(64,0) run concurrently: 371ns/pair vs 826 serial)
  - exp on ScalarE straight out of PSUM (scale = 1/8 folded into exp)
  - PV matmul with stationary [v_h | ones] -> unnormalized out^T plus the
    softmax row-sum in PSUM row 64, in one pass over E
  - final projection consumes the transposed attention output directly
"""

from contextlib import ExitStack
from functools import partial

import ml_dtypes
import numpy as np

import concourse.bass as bass
import concourse.tile as tile
from concourse import bacc, mybir
from concourse import bass_utils

B, S, D = 2, 2048, 1024
HD = 64          # head dim
HPC = 4          # heads per core
E_QK = 512       # q+k columns per core (2 * HPC * HD)
E_V = 256        # v columns per core
NCORES = 8

F32 = mybir.dt.float32
F32R = mybir.dt.float32r
BF16 = mybir.dt.bfloat16

S_TILES = S // 128       # 16
D_TILES = D // 128       # 8
I_CHUNKS = S // 512      # 4 query chunks
J_TILES = S // 128       # 16 key tiles


def _build_nc(reps=1):
    nc = bacc.Bacc("TRN2", target_bir_lowering=False, debug=False, num_devices=NCORES)

    xT = nc.dram_tensor("xT", [D, S], BF16, kind="ExternalInput")
    w_qk = nc.dram_tensor("w_qk", [D, E_QK], BF16, kind="ExternalInput")
    w_v = nc.dram_tensor("w_v", [D, E_V], BF16, kind="ExternalInput")
    w_o = nc.dram_tensor("w_o", [E_V, D], BF16, kind="ExternalInput")
    b_qk = nc.dram_tensor("b_qk", [128, 4], F32, kind="ExternalInput")
    b_v = nc.dram_tensor("b_v", [1, E_V], BF16, kind="ExternalInput")
    ones = nc.dram_tensor("ones", [128, 128], BF16, kind="ExternalInput")
    ones_r = nc.dram_tensor("ones_r", [1, 64], F32R, kind="ExternalInput")
    out = nc.dram_tensor("out", [S, D], BF16, kind="ExternalOutput")

    with tile.TileContext(nc) as tc, ExitStack() as ctx:
        tensors = _alloc(ctx, tc)
        _load_weights(tc, tensors, w_qk.ap(), w_v.ap(), w_o.ap(), b_qk.ap(),
                      b_v.ap(), ones.ap(), ones_r.ap())
        if reps == 1:
            _compute(ctx, tc, tensors, xT.ap(), out.ap())
        else:
            # weights stay resident across iterations; only xT streams in
            with tc.For_i(0, reps) as _i:
                with ExitStack() as ictx:
                    _compute(ictx, tc, tensors, xT.ap(), out.ap())
    nc.compile()
    return nc


def _alloc(ctx, tc):
    persist = ctx.enter_context(tc.tile_pool(name="persist", bufs=1))
    t = {}
    t["xT_sb"] = persist.tile([128, D_TILES, S], BF16, name="xT_sb")
    t["w_qk_sb"] = persist.tile([128, D_TILES, E_QK], BF16, name="w_qk_sb")
    t["w_v_sb"] = persist.tile([128, D_TILES, E_V], BF16, name="w_v_sb")
    t["w_o_sb"] = persist.tile([128, 2, D], BF16, name="w_o_sb")
    t["b_qk_sb"] = persist.tile([128, 4], F32, name="b_qk_sb")
    t["b_v_sb"] = persist.tile([1, E_V], BF16, name="b_v_sb")
    t["ones_sb"] = persist.tile([1, 128], BF16, name="ones_sb")
    t["ones_r_sb"] = persist.tile([1, 64], F32R, name="ones_r_sb")
    t["qkT_sb"] = persist.tile([128, 4, S], BF16, name="qkT_sb")  # q0 q1 k0 k1
    t["v_sb"] = persist.tile([128, S_TILES, HPC, HD + 1], BF16, name="v_sb")
    t["attnT_sb"] = persist.tile([128, 2, S], BF16, name="attnT_sb")
    return t


def _load_weights(tc, t, w_qk, w_v, w_o, b_qk, b_v, ones, ones_r):
    """One-time weight/constant loads (outside the bench rep loop)."""
    nc = tc.nc
    dsl = lambda k: slice(k * 128, (k + 1) * 128)
    for d in range(D_TILES):
        nc.sync.dma_start(t["w_qk_sb"][:, d, :], w_qk[dsl(d), :])
        nc.sync.dma_start(t["w_v_sb"][:, d, :], w_v[dsl(d), :])
    nc.sync.dma_start(t["b_qk_sb"][:], b_qk[:, :])
    nc.sync.dma_start(t["b_v_sb"][:], b_v[:, :])
    nc.sync.dma_start(t["ones_sb"][:], ones[0:1, 0:128])
    nc.sync.dma_start(t["ones_r_sb"][:], ones_r[:, :])
    nc.sync.dma_start(t["v_sb"][:, :, :, HD],
                      ones[:, 0:64].rearrange("p (s h) -> p s h", s=S_TILES))
    for d in range(2):
        nc.sync.dma_start(t["w_o_sb"][:, d, :], w_o[dsl(d), :])


def _compute(ctx, tc, t, xT, out):
    nc = tc.nc
    Exp = mybir.ActivationFunctionType.Exp

    ps_s = ctx.enter_context(tc.tile_pool(name="ps_s", bufs=2, space="PSUM"))
    ps_q = ctx.enter_context(tc.tile_pool(name="ps_q", bufs=2, space="PSUM"))
    ps_pv = ctx.enter_context(tc.tile_pool(name="ps_pv", bufs=2, space="PSUM"))
    epool = ctx.enter_context(tc.tile_pool(name="epool", bufs=6))
    spool = ctx.enter_context(tc.tile_pool(name="spool", bufs=6))
    rpool = ctx.enter_context(tc.tile_pool(name="rpool", bufs=6))

    xT_sb = t["xT_sb"]; w_qk_sb = t["w_qk_sb"]; w_v_sb = t["w_v_sb"]
    w_o_sb = t["w_o_sb"]; b_qk_sb = t["b_qk_sb"]; b_v_sb = t["b_v_sb"]
    ones_sb = t["ones_sb"]; ones_r_sb = t["ones_r_sb"]; qkT_sb = t["qkT_sb"]
    v_sb = t["v_sb"]; attnT_sb = t["attnT_sb"]

    dsl = lambda k: slice(k * 128, (k + 1) * 128)
    ssl = lambda sc: slice(sc * 512, (sc + 1) * 512)

    # ---- per-iteration input DMA: xT chunk sc0 first.  One descriptor
    # per chunk: the ring is issue-limited (~665ns/descriptor), so 8
    # per-tile descriptors would cost 5.3us while the merged transfer is
    # ~3.2us of pure bandwidth. ----
    # sc0 splits its first two d-tiles into a small leading descriptor so
    # the k0 prologue chain starts ~1.5us before the rest of the chunk lands
    nc.sync.dma_start(
        xT_sb[:, 0:2, 0:512],
        xT[0:256, 0:512].rearrange("(t p) c -> p t c", p=128),
    )
    nc.sync.dma_start(
        xT_sb[:, 2:D_TILES, 0:512],
        xT[256:1024, 0:512].rearrange("(t p) c -> p t c", p=128),
    )
    for sc in range(1, I_CHUNKS):
        nc.sync.dma_start(
            xT_sb[:, :, ssl(sc)],
            xT[:, ssl(sc)].rearrange("(t p) c -> p t c", p=128),
        )

    # ---- projection emitters ----
    def emit_qk(et, sc):
        psum = ps_q.tile([128, 512], F32, name="ps_qk", tag="psq")
        for d in range(D_TILES):
            nc.tensor.matmul(
                psum,
                (w_qk_sb[:, d, et * 128:(et + 1) * 128]),
                (xT_sb[:, d, ssl(sc)]),
                start=(d == 0), stop=(d == D_TILES - 1),
            )
        nc.vector.tensor_scalar_add(
            qkT_sb[:, et, ssl(sc)], psum, b_qk_sb[:, et:et + 1],
        )

    def emit_v_chain(st):
        psum = ps_q.tile([128, 512], F32, name="ps_v", tag="psq")[:, :E_V]
        for d in range(D_TILES):
            nc.tensor.matmul(
                psum,
                (xT_sb[:, d, st * 128:(st + 1) * 128]),
                (w_v_sb[:, d, :]),
                start=(d == 0), stop=False,
            )
        # bias via rank-1 ones matmul (K=1)
        nc.tensor.matmul(psum, (ones_sb[:, :]), (b_v_sb[:, :]), start=False, stop=True)
        nc.vector.tensor_copy(
            v_sb[:, st, :, 0:HD],
            psum.rearrange("p (h e) -> p h e", h=HPC),
        )

    def emit_norm(ic, pair, pvA, pvB, tail=False):
        # normalize: 1/rowsum (DVE), broadcast across the 64 hd-partitions
        # via a rank-1 PE matmul (the gpsimd partition_broadcast costs ~2.5us
        # per call and serialized every phase boundary), then multiply.
        isl = ssl(ic)
        recs, pvsbs = [], []
        for pv in (pvA, pvB):
            # approx reciprocal (~18 bits, 1 DVE pass): nc.vector.reciprocal
            # measured 3.3us per [1,512] on HW and serialized every phase
            # boundary.  Rowsums are >= 1e1 so no edge cases.
            sum_sb = rpool.tile([1, 512], F32, name="sum_sb", tag="rec")
            pv_sb = rpool.tile([HD, 512], BF16, name="pv_sb", tag="pvsb")
            if tail:
                # ACT is drained at the very end: stage the psum reads there
                # so the two heads' chains overlap across engines
                nc.scalar.copy(sum_sb[:], pv[HD:HD + 1, :])
                nc.scalar.copy(pv_sb[:], pv[0:HD, :])
            else:
                nc.vector.tensor_copy(sum_sb[:], pv[HD:HD + 1, :])
                nc.vector.tensor_copy(pv_sb[:], pv[0:HD, :])
            rec = rpool.tile([1, 512], F32, name="rec", tag="rec")
            nc.vector.reciprocal_approx_fast(rec[:], sum_sb[:])
            rec_r = rpool.tile([1, 512], F32R, name="rec_r", tag="recr")
            nc.vector.tensor_copy(rec_r[:], rec[:])  # f32r rounding for the PE
            recs.append(rec_r)
            pvsbs.append(pv_sb)
        rbs = []
        for rec_r in recs:
            rb = ps_q.tile([128, 512], F32, name="rb", tag="psq")[0:HD, :]
            nc.tensor.matmul(rb, ones_r_sb[:, :], rec_r[:], start=True, stop=True)
            rbs.append(rb)
        for h_loc in range(2):
            nc.vector.tensor_mul(
                attnT_sb[h_loc * 64:(h_loc + 1) * 64, pair, isl],
                pvsbs[h_loc][:],
                rbs[h_loc][:],
            )

    def emit_final_st(st, mc, last=False):
        # one [128,512] block of the final projection for s-tile st.
        stsl = slice(st * 128, (st + 1) * 128)
        msl = slice(mc * 512, (mc + 1) * 512)
        psF = ps_q.tile([128, 512], F32, name="ps_f", tag="psq")
        for kk in range(2):
            nc.tensor.matmul(
                psF,
                (attnT_sb[:, kk, stsl]),
                (w_o_sb[:, kk, msl]),
                start=(kk == 0), stop=(kk == 1),
            )
        o_t = spool.tile([128, 512], BF16, name="o_t")
        if last:
            # tail: ACT and DVE both idle -- alternate the evacuations
            if (st + mc) % 2 == 0:
                nc.scalar.copy(o_t[:], psF)
            else:
                nc.vector.tensor_copy(o_t[:], psF)
            nc.sync.dma_start(out[stsl, msl], o_t[:])
        else:
            nc.vector.tensor_copy(o_t[:], psF)
            # out-DMA rides the gpsimd HWDGE ring; the sync ring still
            # carries the xT stream at this point
            nc.gpsimd.dma_start(out[stsl, msl], o_t[:])

    # ---- attention step stream ----
    # PE is strictly in-order, so nothing emitted on the PE may wait on the
    # exp that is currently running -- otherwise every filler behind it lands
    # on the ACT critical path.  Per global step t we emit:
    #     exp(t);  S(t+1);  PV(t-1);  [norm when t-1 closed a phase];  fillers
    # PV is one step behind its exp, so its semaphore is always already
    # satisfied when the PE reaches it, and the per-step period settles to
    # max(ACT exp ~1.1us, PE work).
    # pair-0 phases for all ics first, then pair-1: spreads the pair-1
    # projection prerequisites (k1/q1) and the final projections across the
    # whole run instead of piling them into the second phase.
    PHASES = ([(ic, 0) for ic in range(I_CHUNKS)]
              + [(ic, 1) for ic in range(I_CHUNKS)])
    T_STEPS = len(PHASES) * J_TILES
    phase_of = lambda t: (PHASES[t // J_TILES], t % J_TILES)

    def emit_S(t):
        (ic, pair), j = phase_of(t)
        jsl = slice(j * 128, (j + 1) * 128)
        psS = ps_s.tile([128, 1024], F32, name="psS", tag="pss")
        nc.tensor.matmul(
            psS[:, 0:512],
            (qkT_sb[0:64, 2 + pair, jsl]),
            (qkT_sb[0:64, pair, ssl(ic)]),
            start=True, stop=True, tile_position=(0, 0),
        )
        nc.tensor.matmul(
            psS[:, 512:1024],
            (qkT_sb[64:128, 2 + pair, jsl]),
            (qkT_sb[64:128, pair, ssl(ic)]),
            start=True, stop=True, tile_position=(64, 0),
        )
        return psS

    pv_tiles = {}

    def emit_PV(t, e_t):
        (ic, pair), j = phase_of(t)
        p = t // J_TILES
        if j == 0:
            pv_tiles[p] = (
                ps_pv.tile([HD + 1, 512], F32, name="pvA", tag="pv"),
                ps_pv.tile([HD + 1, 512], F32, name="pvB", tag="pv"),
            )
        pvA, pvB = pv_tiles[p]
        nc.tensor.matmul(
            pvA[:], (v_sb[:, j, 2 * pair, :]), (e_t[:, 0:512]),
            start=(j == 0), stop=(j == J_TILES - 1),
        )
        nc.tensor.matmul(
            pvB[:], (v_sb[:, j, 2 * pair + 1, :]), (e_t[:, 512:1024]),
            start=(j == 0), stop=(j == J_TILES - 1),
        )
        if j == J_TILES - 1:
            emit_norm(ic, pair, pvA, pvB, tail=(p == len(PHASES) - 1))

    # ---- filler schedule (global step -> emitters) ----
    qk = lambda et, sc: (lambda: emit_qk(et, sc))
    vch = lambda st: (lambda: emit_v_chain(st))
    fin = lambda st, mc: (lambda: emit_final_st(st, mc))

    # Phase order: idx 0-3 = (ic,0), idx 4-7 = (ic,1).
    # Deadlines: v(st) before its PV at step st+1; k0(sc) before S(4sc) of
    # every pair-0 phase (all in phase 0's steps); q0(ic) before phase idx
    # ic; k1(sc) before phase idx 4's S(4sc); q1(ic) before phase idx 4+ic;
    # final(ic) after norm(ic,1) = end of phase idx 4+ic.
    fillers = {
        # phase (0,0): v chains + k0 chunks just ahead of first use
        0: [vch(0), vch(1)], 1: [vch(2), vch(3)], 2: [qk(2, 1)],
        3: [vch(4), vch(5)], 4: [vch(6), vch(7)], 5: [qk(2, 2)],
        6: [vch(8), vch(9)], 7: [vch(10), vch(11)], 8: [qk(2, 3)],
        9: [vch(12), vch(13)], 10: [vch(14), vch(15)],
        13: [qk(0, 1)],
        # phase (1,0): q0/k1 prefetch
        16 + 4: [qk(0, 2)], 16 + 10: [qk(3, 0)],
        # phase (2,0)
        32 + 4: [qk(0, 3)], 32 + 10: [qk(3, 1)],
        # phase (3,0)
        48 + 2: [qk(3, 2)], 48 + 7: [qk(3, 3)], 48 + 12: [qk(1, 0)],
        # phase (0,1)
        64 + 4: [qk(1, 1)],
        # phase (1,1): final(0) spread
        80 + 4: [qk(1, 2)],
        # phase (2,1)
        96 + 4: [qk(1, 3)],
    }
    for pidx in range(5, 8):
        ic = pidx - 5   # final(ic) fills phase idx pidx (after norm(ic,1))
        base = pidx * J_TILES
        for j in range(8, 16):
            fillers.setdefault(base + j, []).append(
                fin(ic * 4 + (j - 8) // 2, (j - 8) % 2))

    # ---- emission ----
    # prologue: only what S(0)/exp(0) need -- the v chains move into the
    # step-0/1 filler slots so the first exp isn't blocked behind them.
    emit_qk(2, 0)    # k pair 0, keys 0:512
    emit_qk(0, 0)    # q pair 0, queries 0:512

    psS = emit_S(0)
    e_prev = None
    for t in range(T_STEPS):
        e_t = epool.tile([128, 1024], BF16, name="e_t")
        nc.scalar.activation(e_t[:], psS[:], Exp, scale=0.125)
        if t + 1 < T_STEPS:
            psS = emit_S(t + 1)
        if t >= 1:
            emit_PV(t - 1, e_prev)
        e_prev = e_t
        for f in fillers.get(t, ()):
            f()
    emit_PV(T_STEPS - 1, e_prev)
    # tail: last phase's final projection (ACT is idle by now)
    for st in range(12, 16):
        for mc in range(2):
            emit_final_st(st, mc, last=True)


_CACHE = {}


def _get_nc(reps=1):
    key = ("nc", reps)
    if key not in _CACHE:
        _CACHE[key] = _build_nc(reps)
    return _CACHE[key]


def _get_runner(reps=1):
    """Build (once) a jitted shard_map executable over the 8 cores.

    Mirrors bass2jax.run_bass_via_pjrt but caches the jitted function so
    repeat kernel() calls and benchmarking skip retrace/recompile.
    """
    if ("runner", reps) in _CACHE:
        return _CACHE[("runner", reps)]
    import jax
    import jax.numpy as jnp
    from jax.sharding import Mesh, PartitionSpec
    from jax.experimental.shard_map import shard_map
    from concourse import bass2jax

    nc = _get_nc(reps)
    bass2jax.install_neuronx_cc_hook()

    partition_name = nc.partition_id_tensor.name if nc.partition_id_tensor else None
    in_names, out_names, out_avals = [], [], []
    for alloc in nc.m.functions[0].allocations:
        if not isinstance(alloc, mybir.MemoryLocationSet):
            continue
        name = alloc.memorylocations[0].name
        if alloc.kind == "ExternalInput":
            if name != partition_name:
                in_names.append(name)
        elif alloc.kind == "ExternalOutput":
            shape = tuple(alloc.tensor_shape)
            dtype = mybir.dt.np(alloc.dtype)
            out_names.append(name)
            out_avals.append(jax.core.ShapedArray(shape, dtype))
    n_params = len(in_names)
    n_outs = len(out_avals)
    all_names = in_names + out_names
    if partition_name is not None:
        all_names = all_names + [partition_name]
    donate = tuple(range(n_params, n_params + n_outs))

    def _jit_body(*args):
        operands = list(args)
        if partition_name is not None:
            operands.append(bass2jax.partition_id_tensor())
        outs = bass2jax._bass_exec_p.bind(
            *operands,
            out_avals=tuple(out_avals),
            in_names=tuple(all_names),
            out_names=tuple(out_names),
            lowering_input_output_aliases=(),
            sim_require_finite=True,
            sim_require_nnan=True,
            nc=nc,
        )
        return tuple(outs)

    devices = jax.devices()[:NCORES]
    mesh = Mesh(np.asarray(devices), ("core",))
    sharded = jax.jit(
        shard_map(
            _jit_body, mesh=mesh,
            in_specs=(PartitionSpec("core"),) * (n_params + n_outs),
            out_specs=(PartitionSpec("core"),) * n_outs,
            check_rep=False,
        ),
        donate_argnums=donate, keep_unused=True,
    )

    from jax.sharding import NamedSharding
    core_sharding = NamedSharding(mesh, PartitionSpec("core"))

    @partial(jax.jit, out_shardings=core_sharding)
    def _zeros():
        return tuple(
            jnp.zeros((NCORES * a.shape[0],) + a.shape[1:], a.dtype)
            for a in out_avals)

    def run(in_maps, device_arrays=None, timeit=False):
        import time as _time
        if device_arrays is None:
            concat_in = [
                np.concatenate([np.asarray(m[name]) for m in in_maps], axis=0)
                for name in in_names]
            device_arrays = [jax.device_put(a, core_sharding) for a in concat_in]
        zs = jax.block_until_ready(_zeros())
        t0 = _time.perf_counter()
        out_arrs = jax.block_until_ready(sharded(*device_arrays, *zs))
        dt = _time.perf_counter() - t0
        results = [
            {name: np.asarray(out_arrs[i]).reshape(NCORES, *out_avals[i].shape)[c]
             for i, name in enumerate(out_names)}
            for c in range(NCORES)]
        if timeit:
            return results, dt, device_arrays
        return results

    def bench(in_maps, iters=10, batches=3):
        """Pipelined timing: dispatch `iters` executions back-to-back and
        block once, amortizing the per-dispatch RPC latency.  Returns the
        min per-iter average across `batches` batches."""
        import time as _time
        concat_in = [
            np.concatenate([np.asarray(m[name]) for m in in_maps], axis=0)
            for name in in_names]
        device_arrays = [jax.device_put(a, core_sharding) for a in concat_in]
        best = None
        for b in range(batches):
            all_zs = [jax.block_until_ready(_zeros()) for _ in range(iters + 1)]
            jax.block_until_ready(sharded(*device_arrays, *all_zs[0]))
            t0 = _time.perf_counter()
            outs = [sharded(*device_arrays, *all_zs[1 + i]) for i in range(iters)]
            jax.block_until_ready(outs)
            dt = (_time.perf_counter() - t0) / iters
            best = dt if best is None else min(best, dt)
        return best

    _CACHE[("bench", reps)] = bench
    _CACHE[("runner", reps)] = run
    return run


def _core_inputs(x, w_qkv, b_qkv, w_out):
    """Host-side sharding: returns the 8 per-core input dicts (bf16)."""
    bf16 = ml_dtypes.bfloat16
    in_maps = []
    for c in range(NCORES):
        b, g = divmod(c, 4)
        e0 = g * HPC * HD  # first column of this core's head group
        q_cols = slice(e0, e0 + E_V)
        k_cols = slice(D + e0, D + e0 + E_V)
        v_cols = slice(2 * D + e0, 2 * D + e0 + E_V)
        w_qk_c = np.ascontiguousarray(
            np.concatenate([w_qkv[:, q_cols], w_qkv[:, k_cols]], axis=1)).astype(bf16)
        b_qk_c = np.ascontiguousarray(
            np.concatenate([b_qkv[q_cols], b_qkv[k_cols]]).reshape(4, 128).T)
        in_maps.append({
            "xT": np.ascontiguousarray(x[b].T).astype(bf16),
            "w_qk": w_qk_c,
            "w_v": np.ascontiguousarray(w_qkv[:, v_cols]).astype(bf16),
            "w_o": np.ascontiguousarray(w_out[e0:e0 + E_V, :]).astype(bf16),
            "b_qk": b_qk_c,
            "b_v": np.ascontiguousarray(b_qkv[v_cols]).reshape(1, E_V).astype(bf16),
            "ones": np.ones((128, 128), dtype=bf16),
            "ones_r": np.ones((1, 64), dtype=np.float32),
        })
    return in_maps


def kernel(x, w_qkv, b_qkv, w_out, b_out):
    x = np.asarray(x, dtype=np.float32)
    w_qkv = np.asarray(w_qkv, dtype=np.float32)
    b_qkv = np.asarray(b_qkv, dtype=np.float32)
    w_out = np.asarray(w_out, dtype=np.float32)
    b_out = np.asarray(b_out, dtype=np.float32)

    run = _get_runner()
    in_maps = _core_inputs(x, w_qkv, b_qkv, w_out)
    results = run(in_maps)
    partials = np.stack([results[c]["out"].astype(np.float32) for c in range(NCORES)])
    full = partials.reshape(B, 4, S, D).sum(axis=1) + b_out
    return full.astype(np.float32)


# revision 31
# speedup vs baseline: 1.0272x; 1.0272x over previous
"""Trainium2 Bass kernel for nn_AttentionBlock (B=2, S=2048, D=1024, H=16).

Sharding: 8 cores = data-parallel over batch (2) x tensor-parallel over
head groups (4 heads per core).  Each core computes its 4 heads'
attention plus its slice of the qkv / out projections; the host sums the
4 per-batch partial outputs and adds b_out.

All matmul operands and the output DMA are bf16 (PSUM accumulation and
the softmax stay f32); absmax-relative error lands ~7e-3 against the
2e-2 gate.  HW-trace-driven design notes (325us baseline -> ~240us):

  - The kernel is PE-throughput-bound: ~850 matmuls x ~230ns (the PE
    streams 1 moving column/cycle at ~2.0-2.4GHz; LDWEIGHTS overlap).
    ACT's 128 softmax-exp tiles ([128,1024] @ ~1.03us) total ~140us.
  - PE is strictly in-order, so the per-step emission is
    exp(t); S(t+1); PV(t-1); fillers -- the PV lags its exp by one step
    so nothing in the PE stream ever waits on the currently-running exp
    (otherwise every filler chain lands on the ACT critical path).
  - Filler chains (qkv/final projections) ride in the per-step PE slack;
    v/k0 chains are phase-0-locked by dataflow, everything else is
    spread just-ahead-of-deadline.  Phases run pair-0 for all query
    chunks first, then pair-1, which spreads the k1/q1/final work.
  - normalize: DVE reciprocal costs 3.3us per [1,512] -- use
    reciprocal_approx_fast (0.6us) on an SBUF copy of the rowsum (the
    custom DVE op misreads PSUM partition 64 directly), broadcast it
    across the 64 hd-partitions with a rank-1 f32r PE matmul (gpsimd
    partition_broadcast costs ~2.5us), multiply on DVE.
  - weights/consts load once outside the bench rep loop (resident);
    per-iteration DMA is one descriptor per 512-column xT chunk (the
    ring is issue-limited at ~0.66us/descriptor); output rides the
    gpsimd HWDGE ring in bf16.

Per-core layout:
  - host passes x[b].T so the contraction dim (d) is the partition dim
  - q,k computed transposed [e, s]; v computed natural [s, hd]
  - S^T[j,i] = k_h q_h^T, two heads packed in the PE array (row groups
    (0,0)/(64,0) run concurrently: 371ns/pair vs 826 serial)
  - exp on ScalarE straight out of PSUM (scale = 1/8 folded into exp)
  - PV matmul with stationary [v_h | ones] -> unnormalized out^T plus the
    softmax row-sum in PSUM row 64, in one pass over E
  - final projection consumes the transposed attention output directly
"""

from contextlib import ExitStack
from functools import partial

import ml_dtypes
import numpy as np

import concourse.bass as bass
import concourse.tile as tile
from concourse import bacc, mybir
from concourse import bass_utils

B, S, D = 2, 2048, 1024
HD = 64          # head dim
HPC = 4          # heads per core
E_QK = 512       # q+k columns per core (2 * HPC * HD)
E_V = 256        # v columns per core
NCORES = 8

F32 = mybir.dt.float32
F32R = mybir.dt.float32r
BF16 = mybir.dt.bfloat16

S_TILES = S // 128       # 16
D_TILES = D // 128       # 8
I_CHUNKS = S // 512      # 4 query chunks
J_TILES = S // 128       # 16 key tiles


def _build_nc(reps=1):
    nc = bacc.Bacc("TRN2", target_bir_lowering=False, debug=False, num_devices=NCORES)

    xT = nc.dram_tensor("xT", [D, S], BF16, kind="ExternalInput")
    w_qk = nc.dram_tensor("w_qk", [D, E_QK], BF16, kind="ExternalInput")
    w_v = nc.dram_tensor("w_v", [D, E_V], BF16, kind="ExternalInput")
    w_o = nc.dram_tensor("w_o", [E_V, D], BF16, kind="ExternalInput")
    b_qk = nc.dram_tensor("b_qk", [128, 4], F32, kind="ExternalInput")
    b_v = nc.dram_tensor("b_v", [1, E_V], BF16, kind="ExternalInput")
    ones = nc.dram_tensor("ones", [128, 128], BF16, kind="ExternalInput")
    ones_r = nc.dram_tensor("ones_r", [1, 64], F32R, kind="ExternalInput")
    out = nc.dram_tensor("out", [S, D], BF16, kind="ExternalOutput")

    with tile.TileContext(nc) as tc, ExitStack() as ctx:
        tensors = _alloc(ctx, tc)
        _load_weights(tc, tensors, w_qk.ap(), w_v.ap(), w_o.ap(), b_qk.ap(),
                      b_v.ap(), ones.ap(), ones_r.ap())
        if reps == 1:
            _compute(ctx, tc, tensors, xT.ap(), out.ap())
        else:
            # weights stay resident across iterations; only xT streams in
            with tc.For_i(0, reps) as _i:
                with ExitStack() as ictx:
                    _compute(ictx, tc, tensors, xT.ap(), out.ap())
    nc.compile()
    return nc


def _alloc(ctx, tc):
    persist = ctx.enter_context(tc.tile_pool(name="persist", bufs=1))
    t = {}
    t["xT_sb"] = persist.tile([128, D_TILES, S], BF16, name="xT_sb")
    t["w_qk_sb"] = persist.tile([128, D_TILES, E_QK], BF16, name="w_qk_sb")
    t["w_v_sb"] = persist.tile([128, D_TILES, E_V], BF16, name="w_v_sb")
    t["w_o_sb"] = persist.tile([128, 2, D], BF16, name="w_o_sb")
    t["b_qk_sb"] = persist.tile([128, 4], F32, name="b_qk_sb")
    t["b_v_sb"] = persist.tile([1, E_V], BF16, name="b_v_sb")
    t["ones_sb"] = persist.tile([1, 128], BF16, name="ones_sb")
    t["ones_r_sb"] = persist.tile([1, 64], F32R, name="ones_r_sb")
    t["qkT_sb"] = persist.tile([128, 4, S], BF16, name="qkT_sb")  # q0 q1 k0 k1
    t["v_sb"] = persist.tile([128, S_TILES, HPC, HD + 1], BF16, name="v_sb")
    t["attnT_sb"] = persist.tile([128, 2, S], BF16, name="attnT_sb")
    return t


def _load_weights(tc, t, w_qk, w_v, w_o, b_qk, b_v, ones, ones_r):
    """One-time weight/constant loads (outside the bench rep loop)."""
    nc = tc.nc
    dsl = lambda k: slice(k * 128, (k + 1) * 128)
    for d in range(D_TILES):
        nc.sync.dma_start(t["w_qk_sb"][:, d, :], w_qk[dsl(d), :])
        nc.sync.dma_start(t["w_v_sb"][:, d, :], w_v[dsl(d), :])
    nc.sync.dma_start(t["b_qk_sb"][:], b_qk[:, :])
    nc.sync.dma_start(t["b_v_sb"][:], b_v[:, :])
    nc.sync.dma_start(t["ones_sb"][:], ones[0:1, 0:128])
    nc.sync.dma_start(t["ones_r_sb"][:], ones_r[:, :])
    nc.sync.dma_start(t["v_sb"][:, :, :, HD],
                      ones[:, 0:64].rearrange("p (s h) -> p s h", s=S_TILES))
    for d in range(2):
        nc.sync.dma_start(t["w_o_sb"][:, d, :], w_o[dsl(d), :])


def _compute(ctx, tc, t, xT, out):
    nc = tc.nc
    Exp = mybir.ActivationFunctionType.Exp

    ps_s = ctx.enter_context(tc.tile_pool(name="ps_s", bufs=2, space="PSUM"))
    ps_q = ctx.enter_context(tc.tile_pool(name="ps_q", bufs=2, space="PSUM"))
    ps_pv = ctx.enter_context(tc.tile_pool(name="ps_pv", bufs=2, space="PSUM"))
    epool = ctx.enter_context(tc.tile_pool(name="epool", bufs=8))
    spool = ctx.enter_context(tc.tile_pool(name="spool", bufs=6))
    rpool = ctx.enter_context(tc.tile_pool(name="rpool", bufs=6))

    xT_sb = t["xT_sb"]; w_qk_sb = t["w_qk_sb"]; w_v_sb = t["w_v_sb"]
    w_o_sb = t["w_o_sb"]; b_qk_sb = t["b_qk_sb"]; b_v_sb = t["b_v_sb"]
    ones_sb = t["ones_sb"]; ones_r_sb = t["ones_r_sb"]; qkT_sb = t["qkT_sb"]
    v_sb = t["v_sb"]; attnT_sb = t["attnT_sb"]

    dsl = lambda k: slice(k * 128, (k + 1) * 128)
    ssl = lambda sc: slice(sc * 512, (sc + 1) * 512)

    # ---- per-iteration input DMA: xT chunk sc0 first.  One descriptor
    # per chunk: the ring is issue-limited (~665ns/descriptor), so 8
    # per-tile descriptors would cost 5.3us while the merged transfer is
    # ~3.2us of pure bandwidth. ----
    # sc0 splits its first two d-tiles into a small leading descriptor so
    # the k0 prologue chain starts ~1.5us before the rest of the chunk lands
    nc.sync.dma_start(
        xT_sb[:, 0:2, 0:512],
        xT[0:256, 0:512].rearrange("(t p) c -> p t c", p=128),
    )
    nc.sync.dma_start(
        xT_sb[:, 2:D_TILES, 0:512],
        xT[256:1024, 0:512].rearrange("(t p) c -> p t c", p=128),
    )
    for sc in range(1, I_CHUNKS):
        nc.sync.dma_start(
            xT_sb[:, :, ssl(sc)],
            xT[:, ssl(sc)].rearrange("(t p) c -> p t c", p=128),
        )

    # ---- projection emitters ----
    def emit_qk(et, sc):
        psum = ps_q.tile([128, 512], F32, name="ps_qk", tag="psq")
        for d in range(D_TILES):
            nc.tensor.matmul(
                psum,
                (w_qk_sb[:, d, et * 128:(et + 1) * 128]),
                (xT_sb[:, d, ssl(sc)]),
                start=(d == 0), stop=(d == D_TILES - 1),
            )
        nc.vector.tensor_scalar_add(
            qkT_sb[:, et, ssl(sc)], psum, b_qk_sb[:, et:et + 1],
        )

    def emit_v_chain(st):
        psum = ps_q.tile([128, 512], F32, name="ps_v", tag="psq")[:, :E_V]
        for d in range(D_TILES):
            nc.tensor.matmul(
                psum,
                (xT_sb[:, d, st * 128:(st + 1) * 128]),
                (w_v_sb[:, d, :]),
                start=(d == 0), stop=False,
            )
        # bias via rank-1 ones matmul (K=1)
        nc.tensor.matmul(psum, (ones_sb[:, :]), (b_v_sb[:, :]), start=False, stop=True)
        nc.vector.tensor_copy(
            v_sb[:, st, :, 0:HD],
            psum.rearrange("p (h e) -> p h e", h=HPC),
        )

    def emit_norm(ic, pair, pvA, pvB, tail=False):
        # normalize: 1/rowsum (DVE), broadcast across the 64 hd-partitions
        # via a rank-1 PE matmul (the gpsimd partition_broadcast costs ~2.5us
        # per call and serialized every phase boundary), then multiply.
        isl = ssl(ic)
        recs, pvsbs = [], []
        for pv in (pvA, pvB):
            # approx reciprocal (~18 bits, 1 DVE pass): nc.vector.reciprocal
            # measured 3.3us per [1,512] on HW and serialized every phase
            # boundary.  Rowsums are >= 1e1 so no edge cases.
            sum_sb = rpool.tile([1, 512], F32, name="sum_sb", tag="rec")
            pv_sb = rpool.tile([HD, 512], BF16, name="pv_sb", tag="pvsb")
            if tail:
                # ACT is drained at the very end: stage the psum reads there
                # so the two heads' chains overlap across engines
                nc.scalar.copy(sum_sb[:], pv[HD:HD + 1, :])
                nc.scalar.copy(pv_sb[:], pv[0:HD, :])
            else:
                nc.vector.tensor_copy(sum_sb[:], pv[HD:HD + 1, :])
                nc.vector.tensor_copy(pv_sb[:], pv[0:HD, :])
            rec = rpool.tile([1, 512], F32, name="rec", tag="rec")
            nc.vector.reciprocal_approx_fast(rec[:], sum_sb[:])
            rec_r = rpool.tile([1, 512], F32R, name="rec_r", tag="recr")
            nc.vector.tensor_copy(rec_r[:], rec[:])  # f32r rounding for the PE
            recs.append(rec_r)
            pvsbs.append(pv_sb)
        rbs = []
        for rec_r in recs:
            rb = ps_q.tile([128, 512], F32, name="rb", tag="psq")[0:HD, :]
            nc.tensor.matmul(rb, ones_r_sb[:, :], rec_r[:], start=True, stop=True)
            rbs.append(rb)
        for h_loc in range(2):
            nc.vector.tensor_mul(
                attnT_sb[h_loc * 64:(h_loc + 1) * 64, pair, isl],
                pvsbs[h_loc][:],
                rbs[h_loc][:],
            )

    def emit_final_st(st, mc, last=False):
        # one [128,512] block of the final projection for s-tile st.
        stsl = slice(st * 128, (st + 1) * 128)
        msl = slice(mc * 512, (mc + 1) * 512)
        psF = ps_q.tile([128, 512], F32, name="ps_f", tag="psq")
        for kk in range(2):
            nc.tensor.matmul(
                psF,
                (attnT_sb[:, kk, stsl]),
                (w_o_sb[:, kk, msl]),
                start=(kk == 0), stop=(kk == 1),
            )
        o_t = spool.tile([128, 512], BF16, name="o_t")
        if last:
            # tail: ACT and DVE both idle -- alternate the evacuations
            if (st + mc) % 2 == 0:
                nc.scalar.copy(o_t[:], psF)
            else:
                nc.vector.tensor_copy(o_t[:], psF)
            nc.sync.dma_start(out[stsl, msl], o_t[:])
        else:
            nc.vector.tensor_copy(o_t[:], psF)
            # out-DMA rides the gpsimd HWDGE ring; the sync ring still
            # carries the xT stream at this point
            nc.gpsimd.dma_start(out[stsl, msl], o_t[:])

    # ---- attention step stream ----
    # PE is strictly in-order, so nothing emitted on the PE may wait on the
    # exp that is currently running -- otherwise every filler behind it lands
    # on the ACT critical path.  Per global step t we emit:
    #     exp(t);  S(t+1);  PV(t-1);  [norm when t-1 closed a phase];  fillers
    # PV is one step behind its exp, so its semaphore is always already
    # satisfied when the PE reaches it, and the per-step period settles to
    # max(ACT exp ~1.1us, PE work).
    # pair-0 phases for all ics first, then pair-1: spreads the pair-1
    # projection prerequisites (k1/q1) and the final projections across the
    # whole run instead of piling them into the second phase.
    PHASES = ([(ic, 0) for ic in range(I_CHUNKS)]
              + [(ic, 1) for ic in range(I_CHUNKS)])
    T_STEPS = len(PHASES) * J_TILES
    phase_of = lambda t: (PHASES[t // J_TILES], t % J_TILES)

    def emit_S(t):
        (ic, pair), j = phase_of(t)
        jsl = slice(j * 128, (j + 1) * 128)
        psS = ps_s.tile([128, 1024], F32, name="psS", tag="pss")
        nc.tensor.matmul(
            psS[:, 0:512],
            (qkT_sb[0:64, 2 + pair, jsl]),
            (qkT_sb[0:64, pair, ssl(ic)]),
            start=True, stop=True, tile_position=(0, 0),
        )
        nc.tensor.matmul(
            psS[:, 512:1024],
            (qkT_sb[64:128, 2 + pair, jsl]),
            (qkT_sb[64:128, pair, ssl(ic)]),
            start=True, stop=True, tile_position=(64, 0),
        )
        return psS

    pv_tiles = {}

    def emit_PV(t, e_t):
        (ic, pair), j = phase_of(t)
        p = t // J_TILES
        if j == 0:
            pv_tiles[p] = (
                ps_pv.tile([HD + 1, 512], F32, name="pvA", tag="pv"),
                ps_pv.tile([HD + 1, 512], F32, name="pvB", tag="pv"),
            )
        pvA, pvB = pv_tiles[p]
        nc.tensor.matmul(
            pvA[:], (v_sb[:, j, 2 * pair, :]), (e_t[:, 0:512]),
            start=(j == 0), stop=(j == J_TILES - 1),
        )
        nc.tensor.matmul(
            pvB[:], (v_sb[:, j, 2 * pair + 1, :]), (e_t[:, 512:1024]),
            start=(j == 0), stop=(j == J_TILES - 1),
        )
        if j == J_TILES - 1:
            emit_norm(ic, pair, pvA, pvB, tail=(p == len(PHASES) - 1))

    # ---- filler schedule (global step -> emitters) ----
    qk = lambda et, sc: (lambda: emit_qk(et, sc))
    vch = lambda st: (lambda: emit_v_chain(st))
    fin = lambda st, mc: (lambda: emit_final_st(st, mc))

    # Phase order: idx 0-3 = (ic,0), idx 4-7 = (ic,1).
    # Deadlines: v(st) before its PV at step st+1; k0(sc) before S(4sc) of
    # every pair-0 phase (all in phase 0's steps); q0(ic) before phase idx
    # ic; k1(sc) before phase idx 4's S(4sc); q1(ic) before phase idx 4+ic;
    # final(ic) after norm(ic,1) = end of phase idx 4+ic.
    fillers = {
        # phase (0,0): v chains + k0 chunks just ahead of first use
        0: [vch(0), vch(1)], 1: [vch(2), vch(3)], 2: [qk(2, 1)],
        3: [vch(4), vch(5)], 4: [vch(6), vch(7)], 5: [qk(2, 2)],
        6: [vch(8), vch(9)], 7: [vch(10), vch(11)], 8: [qk(2, 3)],
        9: [vch(12), vch(13)], 10: [vch(14), vch(15)],
        13: [qk(0, 1)],
        # phase (1,0): q0/k1 prefetch
        16 + 4: [qk(0, 2)], 16 + 10: [qk(3, 0)],
        # phase (2,0)
        32 + 4: [qk(0, 3)], 32 + 10: [qk(3, 1)],
        # phase (3,0)
        48 + 2: [qk(3, 2)], 48 + 7: [qk(3, 3)], 48 + 12: [qk(1, 0)],
        # phase (0,1)
        64 + 4: [qk(1, 1)],
        # phase (1,1): final(0) spread
        80 + 4: [qk(1, 2)],
        # phase (2,1)
        96 + 4: [qk(1, 3)],
    }
    for pidx in range(5, 8):
        ic = pidx - 5   # final(ic) fills phase idx pidx (after norm(ic,1))
        base = pidx * J_TILES
        for j in range(8, 16):
            fillers.setdefault(base + j, []).append(
                fin(ic * 4 + (j - 8) // 2, (j - 8) % 2))

    # ---- emission ----
    # prologue: only what S(0)/exp(0) need -- the v chains move into the
    # step-0/1 filler slots so the first exp isn't blocked behind them.
    emit_qk(2, 0)    # k pair 0, keys 0:512
    emit_qk(0, 0)    # q pair 0, queries 0:512

    psS = emit_S(0)
    e_prev = None
    for t in range(T_STEPS):
        e_t = epool.tile([128, 1024], BF16, name="e_t")
        nc.scalar.activation(e_t[:], psS[:], Exp, scale=0.125)
        if t + 1 < T_STEPS:
            psS = emit_S(t + 1)
        if t >= 1:
            emit_PV(t - 1, e_prev)
        e_prev = e_t
        for f in fillers.get(t, ()):
            f()
    emit_PV(T_STEPS - 1, e_prev)
    # tail: last phase's final projection (ACT is idle by now)
    for st in range(12, 16):
        for mc in range(2):
            emit_final_st(st, mc, last=True)


_CACHE = {}


def _get_nc(reps=1):
    key = ("nc", reps)
    if key not in _CACHE:
        _CACHE[key] = _build_nc(reps)
    return _CACHE[key]


def _get_runner(reps=1):
    """Build (once) a jitted shard_map executable over the 8 cores.

    Mirrors bass2jax.run_bass_via_pjrt but caches the jitted function so
    repeat kernel() calls and benchmarking skip retrace/recompile.
    """
    if ("runner", reps) in _CACHE:
        return _CACHE[("runner", reps)]
    import jax
    import jax.numpy as jnp
    from jax.sharding import Mesh, PartitionSpec
    from jax.experimental.shard_map import shard_map
    from concourse import bass2jax

    nc = _get_nc(reps)
    bass2jax.install_neuronx_cc_hook()

    partition_name = nc.partition_id_tensor.name if nc.partition_id_tensor else None
    in_names, out_names, out_avals = [], [], []
    for alloc in nc.m.functions[0].allocations:
        if not isinstance(alloc, mybir.MemoryLocationSet):
            continue
        name = alloc.memorylocations[0].name
        if alloc.kind == "ExternalInput":
            if name != partition_name:
                in_names.append(name)
        elif alloc.kind == "ExternalOutput":
            shape = tuple(alloc.tensor_shape)
            dtype = mybir.dt.np(alloc.dtype)
            out_names.append(name)
            out_avals.append(jax.core.ShapedArray(shape, dtype))
    n_params = len(in_names)
    n_outs = len(out_avals)
    all_names = in_names + out_names
    if partition_name is not None:
        all_names = all_names + [partition_name]
    donate = tuple(range(n_params, n_params + n_outs))

    def _jit_body(*args):
        operands = list(args)
        if partition_name is not None:
            operands.append(bass2jax.partition_id_tensor())
        outs = bass2jax._bass_exec_p.bind(
            *operands,
            out_avals=tuple(out_avals),
            in_names=tuple(all_names),
            out_names=tuple(out_names),
            lowering_input_output_aliases=(),
            sim_require_finite=True,
            sim_require_nnan=True,
            nc=nc,
        )
        return tuple(outs)

    devices = jax.devices()[:NCORES]
    mesh = Mesh(np.asarray(devices), ("core",))
    sharded = jax.jit(
        shard_map(
            _jit_body, mesh=mesh,
            in_specs=(PartitionSpec("core"),) * (n_params + n_outs),
            out_specs=(PartitionSpec("core"),) * n_outs,
            check_rep=False,
        ),
        donate_argnums=donate, keep_unused=True,
    )

    from jax.sharding import NamedSharding
    core_sharding = NamedSharding(mesh, PartitionSpec("core"))

    @partial(jax.jit, out_shardings=core_sharding)
    def _zeros():
        return tuple(
            jnp.zeros((NCORES * a.shape[0],) + a.shape[1:], a.dtype)
            for a in out_avals)

    def run(in_maps, device_arrays=None, timeit=False):
        import time as _time
        if device_arrays is None:
            concat_in = [
                np.concatenate([np.asarray(m[name]) for m in in_maps], axis=0)
                for name in in_names]
            device_arrays = [jax.device_put(a, core_sharding) for a in concat_in]
        zs = jax.block_until_ready(_zeros())
        t0 = _time.perf_counter()
        out_arrs = jax.block_until_ready(sharded(*device_arrays, *zs))
        dt = _time.perf_counter() - t0
        results = [
            {name: np.asarray(out_arrs[i]).reshape(NCORES, *out_avals[i].shape)[c]
             for i, name in enumerate(out_names)}
            for c in range(NCORES)]
        if timeit:
            return results, dt, device_arrays
        return results

    def bench(in_maps, iters=10, batches=3):
        """Pipelined timing: dispatch `iters` executions back-to-back and
        block once, amortizing the per-dispatch RPC latency.  Returns the
        min per-iter average across `batches` batches."""
        import time as _time
        concat_in = [
            np.concatenate([np.asarray(m[name]) for m in in_maps], axis=0)
            for name in in_names]
        device_arrays = [jax.device_put(a, core_sharding) for a in concat_in]
        best = None
        for b in range(batches):
            all_zs = [jax.block_until_ready(_zeros()) for _ in range(iters + 1)]
            jax.block_until_ready(sharded(*device_arrays, *all_zs[0]))
            t0 = _time.perf_counter()
            outs = [sharded(*device_arrays, *all_zs[1 + i]) for i in range(iters)]
            jax.block_until_ready(outs)
            dt = (_time.perf_counter() - t0) / iters
            best = dt if best is None else min(best, dt)
        return best

    _CACHE[("bench", reps)] = bench
    _CACHE[("runner", reps)] = run
    return run


def _core_inputs(x, w_qkv, b_qkv, w_out):
    """Host-side sharding: returns the 8 per-core input dicts (bf16)."""
    bf16 = ml_dtypes.bfloat16
    in_maps = []
    for c in range(NCORES):
        b, g = divmod(c, 4)
        e0 = g * HPC * HD  # first column of this core's head group
        q_cols = slice(e0, e0 + E_V)
        k_cols = slice(D + e0, D + e0 + E_V)
        v_cols = slice(2 * D + e0, 2 * D + e0 + E_V)
        w_qk_c = np.ascontiguousarray(
            np.concatenate([w_qkv[:, q_cols], w_qkv[:, k_cols]], axis=1)).astype(bf16)
        b_qk_c = np.ascontiguousarray(
            np.concatenate([b_qkv[q_cols], b_qkv[k_cols]]).reshape(4, 128).T)
        in_maps.append({
            "xT": np.ascontiguousarray(x[b].T).astype(bf16),
            "w_qk": w_qk_c,
            "w_v": np.ascontiguousarray(w_qkv[:, v_cols]).astype(bf16),
            "w_o": np.ascontiguousarray(w_out[e0:e0 + E_V, :]).astype(bf16),
            "b_qk": b_qk_c,
            "b_v": np.ascontiguousarray(b_qkv[v_cols]).reshape(1, E_V).astype(bf16),
            "ones": np.ones((128, 128), dtype=bf16),
            "ones_r": np.ones((1, 64), dtype=np.float32),
        })
    return in_maps


def kernel(x, w_qkv, b_qkv, w_out, b_out):
    x = np.asarray(x, dtype=np.float32)
    w_qkv = np.asarray(w_qkv, dtype=np.float32)
    b_qkv = np.asarray(b_qkv, dtype=np.float32)
    w_out = np.asarray(w_out, dtype=np.float32)
    b_out = np.asarray(b_out, dtype=np.float32)

    run = _get_runner()
    in_maps = _core_inputs(x, w_qkv, b_qkv, w_out)
    results = run(in_maps)
    partials = np.stack([results[c]["out"].astype(np.float32) for c in range(NCORES)])
    full = partials.reshape(B, 4, S, D).sum(axis=1) + b_out
    return full.astype(np.float32)
